# revision 14
# baseline (speedup 1.0000x reference)
"""AKT dense-transformer Bass kernel for 8 trn2 NeuronCores.

Sharding: data-parallel over batch (cores 0-3 = batch 0, cores 4-7 = batch 1).
Within a group each lane owns interleaved query row-blocks rg = 4*r + lane
(r = 0..3), so the causal-truncation width per local block r is
W_r = 512*(r+1), uniform across lanes (SPMD-safe).  Layer outputs are
republished group-wide with a transposed bf16 AllGather.  Matmuls run in bf16
(fp32 PSUM accumulation); the AKT softmax/decay chain runs in fp32 on DVE/ACT
using tensor_tensor_scan for the cumulative sum.

Falls back to a NumPy implementation if anything in the Bass path fails or the
inputs violate the fast path's assumptions (nonzero biases, non-unit LN).
"""

import os
import sys
import traceback

import numpy as np

B, S, D, H, DFF, L = 2, 2048, 512, 8, 2048, 3
DK = D // H  # 64
NLB = 4      # local row-blocks per core
P = 128

ORDER = ["s_embed_data", "sa_embed_data", "Wk", "bk", "Wv", "bv", "Wo", "bo",
         "gammas", "ln1_s", "ln1_b", "W1", "b1", "W2", "b2", "ln2_s", "ln2_b"]

# ---------------------------------------------------------------------------
# NumPy fallback
# ---------------------------------------------------------------------------

def _softmax(x, axis=-1):
    m = np.max(x, axis=axis, keepdims=True)
    e = np.exp(x - m)
    return e / np.sum(e, axis=axis, keepdims=True)


def _ln_np(x, s, b):
    m = x.mean(-1, keepdims=True)
    v = ((x - m) ** 2).mean(-1, keepdims=True)
    return (x - m) / np.sqrt(v + 1e-5) * s + b


_POS = None


def _pos():
    global _POS
    if _POS is None:
        ar = np.arange(S, dtype=np.float32)
        _POS = np.abs(ar[None, :] - ar[:, None])
    return _POS


def _attn_np(q, k, v, mask, zero_pad, gamma):
    scores = (q @ k.T) / np.float32(np.sqrt(DK))
    scores_ = _softmax(scores, axis=-1) * mask
    distcum = np.cumsum(scores_, axis=-1)
    disttotal = np.sum(scores_, axis=-1, keepdims=True)
    dist = np.sqrt(np.clip((disttotal - distcum) * _pos(), 0.0, None),
                   dtype=np.float32)
    g = -np.logaddexp(0.0, gamma)
    te = np.clip(np.exp(dist * g), 1e-5, 1e5).astype(np.float32)
    scores = scores * te
    scores = np.where(mask, scores, np.float32(-1e32))
    scores = _softmax(scores, axis=-1)
    if zero_pad:
        scores[0, :] = 0.0
    return scores @ v


def _layer_np(i, mask_k, query, keyx, values, apply_pos, a):
    q_all = query @ a["Wk"][i] + a["bk"][i]
    k_all = keyx @ a["Wk"][i] + a["bk"][i]
    v_all = values @ a["Wv"][i] + a["bv"][i]
    mask = np.tril(np.ones((S, S), bool), k=0 if mask_k == 1 else -1)
    o = np.empty((B, S, D), np.float32)
    for b in range(B):
        for h in range(H):
            sl = slice(h * DK, (h + 1) * DK)
            o[b, :, sl] = _attn_np(q_all[b, :, sl], k_all[b, :, sl],
                                   v_all[b, :, sl], mask, mask_k == 0,
                                   float(a["gammas"][i, h, 0, 0]))
    o = o @ a["Wo"][i] + a["bo"][i]
    out = _ln_np(query + o, a["ln1_s"][i], a["ln1_b"][i])
    if apply_pos:
        ff = np.maximum(out @ a["W1"][i] + a["b1"][i], 0.0) @ a["W2"][i] + a["b2"][i]
        out = _ln_np(out + ff, a["ln2_s"][i], a["ln2_b"][i])
    return out.astype(np.float32)


def _kernel_numpy(a):
    y = _layer_np(0, 1, a["sa_embed_data"], a["sa_embed_data"],
                  a["sa_embed_data"], True, a)
    x = _layer_np(1, 1, a["s_embed_data"], a["s_embed_data"],
                  a["s_embed_data"], False, a)
    x = _layer_np(2, 0, x, x, y, True, a)
    return x.astype(np.float32)


# ---------------------------------------------------------------------------
# Bass fast path
# ---------------------------------------------------------------------------

_NC = None


def _build_bass():
    import concourse.bass as bass
    import concourse.mybir as mybir
    import concourse.tile as tile

    # The Drain CTRL instruction supports only one sem wait on this walrus;
    # split the TileContext exit drain into one drain per wait.
    def _patched_drain_and_barrier(self, tick_clock, wait_clock):
        nc = self.nc
        drain_inst = nc.sync.drain()
        wait_clock.add_sem_waits(
            drain_inst.ins, tile.ScopedClock({None: tick_clock.global_clock}))
        si = drain_inst.ins.sync_info
        if si is not None and si.on_wait is not None and len(si.on_wait) > 1:
            waits = list(si.on_wait)
            si.on_wait = waits[:1]
            for w in waits[1:]:
                d2 = nc.sync.drain()
                si2 = d2.ins.sync_info
                if si2 is None:
                    d2.ins.sync_info = type(drain_inst.ins.sync_info)(
                        on_update=[], on_wait=[w])
                else:
                    si2.on_wait = [w]
        nc.all_engine_barrier()
        assert self.sems is not None
        popped = nc._tile_sem_poison_stack.pop()
        assert popped is self._sem_poison
        nc.clear_and_free_semaphores(list(self.sems.allocated().values()))
        nc.all_engine_barrier()

    tile.TileContext._drain_and_barrier = _patched_drain_and_barrier

    def _split_waits(nc):
        """walrus here allows only one sem-wait per instruction; hoist
        extras onto same-engine NoOps inserted just before."""
        n = 0
        for bb in nc.main_func.blocks:
            insts = bb.bb.instructions if hasattr(bb, "bb") else bb.instructions
            i = 0
            while i < len(insts):
                inst = insts[i]
                si = getattr(inst, "sync_info", None)
                if si is not None and si.on_wait is not None and len(si.on_wait) > 1:
                    waits = list(si.on_wait)
                    si.on_wait = waits[-1:]
                    for j, w in enumerate(waits[:-1]):
                        n += 1
                        nop = mybir.InstNoOp(
                            name=f"{inst.name}_wsplit{j}",
                            engine=inst.engine,
                            sync_info=mybir.SyncInfo(on_wait=[w], on_update=[]),
                            bass_nofuse=True,
                        )
                        insts.insert(i, nop)
                        i += 1
                i += 1
        return n

    dt = mybir.dt
    A = mybir.AluOpType
    AF = mybir.ActivationFunctionType
    f32, bf16 = dt.float32, dt.bfloat16

    nc = bass.Bass("TRN2", target_bir_lowering=False, debug=False,
                   num_devices=8)

    def din(name, shape, dty=f32):
        return nc.dram_tensor(name, shape, dty, kind="ExternalInput")

    saTo = din("saTo", [D, 512], bf16)
    sTo = din("sTo", [D, 512], bf16)
    sa_rows = din("sa_rows", [512, D])
    s_rows = din("s_rows", [512, D])
    # lane slices of the stacked replicated tensors (AllGathered on device)
    w5_sl = din("w5_sl", [2176, D], bf16)    # [wk0-2;wv0-2;wo0-2;w2_0;w2_2]/4
    w1_sl = din("w1_sl", [512, DFF], bf16)   # [w1_0;w1_2;saT;sT]/4
    g_in = din("gbc", [P, 24])
    mb_in = din("mbc", [P, 12])
    zp_in = din("zp", [1, 1])
    idb_in = din("idb", [P, P], bf16)
    idf_in = din("idf", [P, P])

    out_rows = nc.dram_tensor("out_rows", [512, D], f32, kind="ExternalOutput")

    RG = [[0, 1, 2, 3], [4, 5, 6, 7]]

    with tile.TileContext(nc) as tc:
        with (
            tc.tile_pool(name="c1", bufs=1) as c1,
            tc.tile_pool(name="xs", bufs=4) as xs,      # X^T source tiles
            tc.tile_pool(name="kt", bufs=4) as ktp,     # K^T tiles
            tc.tile_pool(name="ba", bufs=1) as ba,      # A/B/C big blocks
            tc.tile_pool(name="bp", bufs=1) as bp,      # P
            tc.tile_pool(name="fr", bufs=4) as fr,      # small frags
            tc.tile_pool(name="ro", bufs=1) as ro,      # row tiles
            tc.tile_pool(name="sm", bufs=8) as smp,     # [128,1] scalars
            tc.tile_pool(name="pss", bufs=1, space="PSUM") as pss,
            tc.tile_pool(name="pst", bufs=1, space="PSUM") as pst,
            tc.tile_pool(name="pso", bufs=1, space="PSUM") as pso,
            tc.tile_pool(name="psw", bufs=2, space="PSUM") as psw,
            tc.tile_pool(name="dram", bufs=2, space="DRAM") as dram,
        ):

            _tcnt = [0]

            def T(pool, shape, dty, tag):
                _tcnt[0] += 1
                return pool.tile(shape, dty, name=f"{tag}_{_tcnt[0]}", tag=tag)

            t0 = T(c1, [P, S], f32, "t0")
            nc.gpsimd.iota(t0[:], pattern=[[-1, S]], base=0,
                           channel_multiplier=1,
                           allow_small_or_imprecise_dtypes=True)
            gbc = T(c1, [P, 24], f32, "gbc")
            nc.sync.dma_start(gbc[:], g_in[:])
            mbc = T(c1, [P, 12], f32, "mbc")
            nc.sync.dma_start(mbc[:], mb_in[:])
            zp = T(c1, [1, 1], f32, "zp")
            nc.sync.dma_start(zp[:], zp_in[:])
            idb = T(c1, [P, P], bf16, "idb")
            nc.sync.dma_start(idb[:], idb_in[:])
            idf = T(c1, [P, P], f32, "idf")
            nc.sync.dma_start(idf[:], idf_in[:])

            eps = T(c1, [P, 1], f32, "eps")
            nc.gpsimd.memset(eps[:], 1e-5)
            w5b = dram.tile([2176, D], bf16, name="w5b", tag="w5b")
            nc.gpsimd.dma_start(w5b[:], w5_sl[:])
            w5g = dram.tile([8704, D], bf16, name="w5g", tag="w5g")
            nc.gpsimd.collective_compute(
                "AllGather", A.bypass, replica_groups=RG,
                ins=[w5b.opt()], outs=[w5g.opt()])
            w1b = dram.tile([512, DFF], bf16, name="w1b", tag="w1b")
            nc.gpsimd.dma_start(w1b[:], w1_sl[:])
            w1g = dram.tile([2048, DFF], bf16, name="w1g", tag="w1g")
            nc.gpsimd.collective_compute(
                "AllGather", A.bypass, replica_groups=RG,
                ins=[w1b.opt()], outs=[w1g.opt()])

            x1_rows = [T(c1, [P, D], f32, f"x1r{r}") for r in range(NLB)]
            x1T = [T(c1, [P, 512], bf16, f"x1T{i}") for i in range(4)]

            def tr_f32(dst_ap, src_ap):
                pt = T(pst, [P, P], f32, "pt")
                nc.tensor.transpose(pt[:], src_ap, idf[:])
                nc.vector.tensor_copy(dst_ap, pt[:])

            def ln_tile(xr, out_tag):
                s1 = T(smp, [P, 1], f32, "ls1")
                nc.vector.tensor_reduce(s1[:], xr[:],
                                        axis=mybir.AxisListType.X, op=A.add)
                nm = T(smp, [P, 1], f32, "lnm")
                nc.scalar.mul(nm[:], s1[:], -1.0 / D)
                xc = T(ro, [P, D], f32, "xc")
                nc.scalar.activation(xc[:], xr[:], AF.Identity, bias=nm[:])
                sq = T(psw, [P, D], f32, "w")
                s2_ = T(smp, [P, 1], f32, "ls2")
                nc.scalar.activation(sq[:], xc[:], AF.Square,
                                     accum_out=s2_[:])
                sd = T(smp, [P, 1], f32, "lsd")
                nc.scalar.activation(sd[:], s2_[:], AF.Sqrt, scale=1.0 / D,
                                     bias=eps[:])
                rs = T(smp, [P, 1], f32, "lrs")
                nc.vector.reciprocal(rs[:], sd[:])
                o = T(ro, [P, D], f32, out_tag)
                nc.vector.tensor_scalar(o[:], xc[:], rs[:], None, op0=A.mult)
                return o

            def layer(l, strict, v_xt, q_xt, qto, rows_in, apply_pos):
                # per-layer weights
                wk = [T(c1, [P, D], bf16, f"wk{i}") for i in range(4)]
                wv = [T(c1, [P, D], bf16, f"wv{i}") for i in range(4)]
                wo = [T(c1, [P, D], bf16, f"wo{i}") for i in range(4)]
                for i in range(4):
                    nc.sync.dma_start(
                        wk[i][:], w5g[l * 512 + i * P:l * 512 + (i + 1) * P, :])
                    nc.sync.dma_start(
                        wv[i][:],
                        w5g[(3 + l) * 512 + i * P:(3 + l) * 512 + (i + 1) * P, :])
                    nc.sync.dma_start(
                        wo[i][:],
                        w5g[(6 + l) * 512 + i * P:(6 + l) * 512 + (i + 1) * P, :])

                # V = Xv @ Wv : 16 k-block tiles [128, 512] bf16
                vt = []
                for kb in range(16):
                    ps = T(psw, [P, D], f32, "w")
                    for dc in range(4):
                        nc.tensor.matmul(
                            ps[:], v_xt[dc][:, kb * P:(kb + 1) * P], wv[dc][:],
                            start=(dc == 0), stop=(dc == 3))
                    v_ = T(c1, [P, D], bf16, f"v{kb}")
                    nc.scalar.copy(v_[:], ps[:])
                    vt.append(v_)

                # K^T = Wk^T @ Xq^T : 4 tiles [128, 2048] bf16
                kt = []
                for kc in range(4):
                    ps = T(pss, [P, S], f32, "s")
                    for dc in range(4):
                        for b4 in range(4):
                            nc.tensor.matmul(
                                ps[:, b4 * 512:(b4 + 1) * 512],
                                wk[dc][:, kc * P:(kc + 1) * P],
                                q_xt[dc][:, b4 * 512:(b4 + 1) * 512],
                                start=(dc == 0), stop=(dc == 3))
                    k_ = T(ktp, [P, S], bf16, "kt")
                    nc.vector.tensor_copy(k_[:], ps[:])
                    kt.append(k_)

                # Q^T own rows: 4 tiles [128, 512] bf16
                qtm = []
                for kc in range(4):
                    ps = T(psw, [P, D], f32, "w")
                    for dc in range(4):
                        nc.tensor.matmul(
                            ps[:], wk[dc][:, kc * P:(kc + 1) * P], qto[dc][:],
                            start=(dc == 0), stop=(dc == 3))
                    q_ = T(c1, [P, D], bf16, f"qtm{kc}")
                    nc.scalar.copy(q_[:], ps[:])
                    qtm.append(q_)

                oT = [T(c1, [P, 512], bf16, f"oT{i}") for i in range(4)]
                mboff = 4 if strict else 0
                for h in range(H):
                    ht, hp = h // 2, (h % 2) * DK
                    g_ap = gbc[:, (l * 8 + h):(l * 8 + h + 1)]
                    for r in range(NLB):
                        W_ = 512 * (r + 1)
                        nch = 4 * (r + 1)
                        mb_ap = mbc[:, mboff + r:mboff + r + 1]
                        rb_ap = mbc[:, 8 + r:8 + r + 1]

                        ps = T(pss, [P, S], f32, "s")
                        for b4 in range(4):
                            nc.tensor.matmul(
                                ps[:, b4 * 512:(b4 + 1) * 512],
                                qtm[ht][hp:hp + DK, r * P:(r + 1) * P],
                                kt[ht][hp:hp + DK, b4 * 512:(b4 + 1) * 512],
                                start=True, stop=True)

                        at = T(ba, [P, S], f32, "A")
                        bt = T(ba, [P, S], f32, "Bt")
                        ct = T(ba, [P, S], f32, "C")
                        z1 = T(smp, [P, 1], f32, "z1")
                        iz1 = T(smp, [P, 1], f32, "iz1")
                        z2 = T(smp, [P, 1], f32, "z2")
                        iz2 = T(smp, [P, 1], f32, "iz2")

                        nc.scalar.activation(at[:], ps[:], AF.Exp,
                                             scale=0.125, accum_out=z1[:])
                        nc.vector.reciprocal(iz1[:], z1[:])
                        nc.vector.scalar_tensor_tensor(
                            bt[:, :W_], t0[:, :W_], mb_ap, at[:, :W_],
                            op0=A.is_ge, op1=A.mult)
                        nc.vector.tensor_tensor_scan(
                            ct[:, :W_], bt[:, :W_], bt[:, :W_], 0.0,
                            op0=A.add, op1=A.max)
                        nc.vector.tensor_scalar(
                            bt[:, :W_], ct[:, :W_], ct[:, W_ - 1:W_], -1.0,
                            op0=A.subtract, op1=A.mult)
                        nc.vector.scalar_tensor_tensor(
                            ct[:, :W_], t0[:, :W_], rb_ap, bt[:, :W_],
                            op0=A.add, op1=A.mult)
                        nc.scalar.activation(bt[:, :W_], ct[:, :W_], AF.Sqrt,
                                             scale=iz1[:])
                        nc.scalar.activation(ct[:, :W_], bt[:, :W_], AF.Exp,
                                             scale=g_ap)
                        nc.vector.tensor_tensor(
                            bt[:, :W_], ps[:, :W_], ct[:, :W_], op=A.mult)
                        nc.scalar.activation(ct[:, :W_], bt[:, :W_], AF.Exp,
                                             scale=0.125)
                        nc.vector.scalar_tensor_tensor(
                            bt[:, :W_], t0[:, :W_], mb_ap, ct[:, :W_],
                            op0=A.is_ge, op1=A.mult, accum_out=z2[:])
                        nc.vector.tensor_scalar(
                            z2[:], z2[:], 1e-30, None, op0=A.add)
                        nc.vector.reciprocal(iz2[:], z2[:])
                        pb = T(bp, [P, S], bf16, "P")
                        nc.vector.tensor_scalar(
                            pb[:, :W_], bt[:, :W_], iz2[:], None, op0=A.mult)
                        if strict and r == 0:
                            nc.vector.tensor_scalar(
                                pb[0:1, :W_], pb[0:1, :W_], zp[:], None,
                                op0=A.mult)

                        po = T(pso, [DK, P], f32, "o")
                        for kc in range(nch):
                            pf = T(pst, [P, P], bf16, "pt")
                            nc.tensor.transpose(
                                pf[:], pb[:, kc * P:(kc + 1) * P], idb[:])
                            pfs = T(fr, [P, P], bf16, "pf")
                            nc.scalar.copy(pfs[:], pf[:])
                            nc.tensor.matmul(
                                po[:], vt[kc][:, h * DK:(h + 1) * DK], pfs[:],
                                start=(kc == 0), stop=(kc == nch - 1))
                        nc.vector.tensor_copy(
                            oT[ht][hp:hp + DK, r * P:(r + 1) * P], po[:])

                out_r = []
                for r in range(NLB):
                    ps = T(psw, [P, D], f32, "w")
                    for ch in range(4):
                        nc.tensor.matmul(
                            ps[:], oT[ch][:, r * P:(r + 1) * P], wo[ch][:],
                            start=(ch == 0), stop=(ch == 3))
                    xr = T(ro, [P, D], f32, "xr")
                    nc.vector.tensor_tensor(xr[:], ps[:], rows_in[r][:],
                                            op=A.add)
                    out_r.append(ln_tile(xr, f"lo{r}"))

                if not apply_pos:
                    return out_r

                w1 = [T(c1, [P, DFF], bf16, f"w1{i}") for i in range(4)]
                w2 = [T(c1, [P, D], bf16, f"w2{i}") for i in range(16)]
                w1base = 0 if l == 0 else 512
                w2base = 4608 if l == 0 else 6656
                for i in range(4):
                    nc.sync.dma_start(
                        w1[i][:], w1g[w1base + i * P:w1base + (i + 1) * P, :])
                for i in range(16):
                    nc.sync.dma_start(
                        w2[i][:], w5g[w2base + i * P:w2base + (i + 1) * P, :])
                fx = [T(c1, [P, 512], bf16, f"fxT{i}") for i in range(4)]
                for r in range(NLB):
                    for dc in range(4):
                        tr_f32(fx[dc][:, r * P:(r + 1) * P],
                               out_r[r][:, dc * P:(dc + 1) * P])
                fin = []
                for r in range(NLB):
                    po2 = T(pso, [P, D], f32, "o")
                    first = True
                    for b4 in range(4):
                        ph = T(psw, [P, 512], f32, "w")
                        for dc in range(4):
                            nc.tensor.matmul(
                                ph[:], fx[dc][:, r * P:(r + 1) * P],
                                w1[dc][:, b4 * 512:(b4 + 1) * 512],
                                start=(dc == 0), stop=(dc == 3))
                        hh = T(fr, [P, 512], bf16, "hh")
                        nc.scalar.activation(hh[:], ph[:], AF.Relu)
                        for ff in range(4):
                            pf = T(pst, [P, P], bf16, "pt")
                            nc.tensor.transpose(
                                pf[:], hh[:, ff * P:(ff + 1) * P], idb[:])
                            hf = T(fr, [P, P], bf16, "pf")
                            nc.scalar.copy(hf[:], pf[:])
                            nc.tensor.matmul(
                                po2[:], hf[:], w2[b4 * 4 + ff][:],
                                start=first, stop=(b4 == 3 and ff == 3))
                            first = False
                    xr2 = T(ro, [P, D], f32, "xr2")
                    nc.vector.tensor_tensor(xr2[:], po2[:], out_r[r][:],
                                            op=A.add)
                    fin.append(ln_tile(xr2, f"lf{r}"))
                return fin

            def publish(row_tiles, tag):
                ot = [T(c1, [P, 512], bf16, f"pub{tag}{i}")
                      for i in range(4)]
                for r in range(NLB):
                    for dc in range(4):
                        tr_f32(ot[dc][:, r * P:(r + 1) * P],
                               row_tiles[r][:, dc * P:(dc + 1) * P])
                bin_ = T(dram, [D, 512], bf16, f"bi{tag}")
                for dc in range(4):
                    nc.gpsimd.dma_start(bin_[dc * P:(dc + 1) * P, :],
                                        ot[dc][:])
                bout = T(dram, [4 * D, 512], bf16, f"bo{tag}")
                nc.gpsimd.collective_compute(
                    "AllGather", A.bypass, replica_groups=RG,
                    ins=[bin_.opt()], outs=[bout.opt()])
                return bout, ot

            def gath_to_xt(bout):
                xts = []
                for dc in range(4):
                    xt_t = T(xs, [P, S], bf16, "xsrc")
                    for r2 in range(4):
                        for c2 in range(4):
                            nc.sync.dma_start(
                                xt_t[:, (4 * r2 + c2) * P:(4 * r2 + c2 + 1) * P],
                                bout[c2 * 512 + dc * P:c2 * 512 + (dc + 1) * P,
                                     r2 * P:(r2 + 1) * P])
                    xts.append(xt_t)
                return xts

            # ---------------- main ----------------
            sa_r = [T(ro, [P, D], f32, f"rin{r}") for r in range(NLB)]
            for r in range(NLB):
                nc.sync.dma_start(sa_r[r][:], sa_rows[r * P:(r + 1) * P, :])
            saT_t = [T(xs, [P, S], bf16, "xsrc") for _ in range(4)]
            for dc in range(4):
                nc.sync.dma_start(
                    saT_t[dc][:], w1g[1024 + dc * P:1024 + (dc + 1) * P, :])
            qto0 = [T(c1, [P, 512], bf16, f"qto{i}") for i in range(4)]
            for dc in range(4):
                nc.sync.dma_start(qto0[dc][:], saTo[dc * P:(dc + 1) * P, :])

            y_rows = layer(0, False, saT_t, saT_t, qto0, sa_r, True)
            y_g, _ = publish(y_rows, "y")

            s_r = [T(ro, [P, D], f32, f"rin{r}") for r in range(NLB)]
            for r in range(NLB):
                nc.sync.dma_start(s_r[r][:], s_rows[r * P:(r + 1) * P, :])
            sT_t = [T(xs, [P, S], bf16, "xsrc") for _ in range(4)]
            for dc in range(4):
                nc.sync.dma_start(
                    sT_t[dc][:], w1g[1536 + dc * P:1536 + (dc + 1) * P, :])
            qto1 = [T(c1, [P, 512], bf16, f"qto{i}") for i in range(4)]
            for dc in range(4):
                nc.sync.dma_start(qto1[dc][:], sTo[dc * P:(dc + 1) * P, :])

            x_rows = layer(1, False, sT_t, sT_t, qto1, s_r, False)
            for r in range(NLB):
                nc.vector.tensor_copy(x1_rows[r][:], x_rows[r][:])
            x_g, x_ot = publish(x_rows, "x")
            for dc in range(4):
                nc.vector.tensor_copy(x1T[dc][:], x_ot[dc][:])

            yT_t = gath_to_xt(y_g)
            xT_t = gath_to_xt(x_g)
            f_rows = layer(2, True, yT_t, xT_t, x1T, x1_rows, True)
            for r in range(NLB):
                nc.sync.dma_start(out_rows[r * P:(r + 1) * P, :], f_rows[r][:])

    _split_waits(nc)
    return nc


def _prep_inputs(a):
    import ml_dtypes
    bf16 = ml_dtypes.bfloat16

    g = -np.logaddexp(0.0, a["gammas"][:, :, 0, 0]).astype(np.float32)
    gb = np.zeros((128, 24), np.float32)
    for l in range(3):
        for h in range(8):
            gb[:, l * 8 + h] = g[l, h]
    ident = np.eye(128, dtype=np.float32)

    w5 = np.concatenate([a["Wk"][0], a["Wk"][1], a["Wk"][2],
                         a["Wv"][0], a["Wv"][1], a["Wv"][2],
                         a["Wo"][0], a["Wo"][1], a["Wo"][2],
                         a["W2"][0], a["W2"][2]], axis=0).astype(bf16)

    in_maps = []
    for core in range(8):
        b, lane = core // 4, core % 4
        rows_idx = np.concatenate(
            [np.arange((4 * r + lane) * 128, (4 * r + lane) * 128 + 128)
             for r in range(4)])
        sa_b = a["sa_embed_data"][b]
        s_b = a["s_embed_data"][b]
        mb = np.zeros((128, 12), np.float32)
        for r in range(4):
            rb = (4 * r + lane) * 128.0
            mb[:, r] = -rb
            mb[:, 4 + r] = -rb + 1.0
            mb[:, 8 + r] = rb
        w1stack = np.concatenate(
            [a["W1"][0], a["W1"][2],
             np.ascontiguousarray(sa_b.T), np.ascontiguousarray(s_b.T)],
            axis=0).astype(bf16)
        m = {
            "w5_sl": w5[lane * 2176:(lane + 1) * 2176],
            "w1_sl": np.ascontiguousarray(
                w1stack[lane * 512:(lane + 1) * 512]),
            "saTo": np.ascontiguousarray(sa_b[rows_idx].T).astype(bf16),
            "sTo": np.ascontiguousarray(s_b[rows_idx].T).astype(bf16),
            "sa_rows": np.ascontiguousarray(sa_b[rows_idx]),
            "s_rows": np.ascontiguousarray(s_b[rows_idx]),
            "gbc": gb,
            "mbc": mb,
            "zp": np.array([[0.0 if lane == 0 else 1.0]], np.float32),
            "idb": ident.astype(bf16),
            "idf": ident,
        }
        in_maps.append(m)
    return in_maps


def _run_fast(nc, a):
    """Fork of bass2jax.run_bass_via_pjrt's 8-core path that overlaps the
    host->device input transfer (background thread) with XLA/walrus compile
    (AOT .lower().compile() needs only avals)."""
    import threading
    import jax
    import jax.numpy as jnp
    from jax.sharding import Mesh, PartitionSpec, NamedSharding
    from jax.experimental.shard_map import shard_map
    import concourse.mybir as mybir
    from concourse import bass2jax

    bass2jax.install_neuronx_cc_hook()
    n_cores = 8
    devices = jax.devices()[:n_cores]
    assert len(devices) == n_cores
    mesh = Mesh(np.asarray(devices), ("core",))
    sh = NamedSharding(mesh, PartitionSpec("core"))

    in_names, out_names, out_avals = [], [], []
    zero_shapes = []
    for alloc in nc.m.functions[0].allocations:
        if not isinstance(alloc, mybir.MemoryLocationSet):
            continue
        name = alloc.memorylocations[0].name
        if alloc.kind == "ExternalInput":
            in_names.append(name)
        elif alloc.kind == "ExternalOutput":
            out_names.append(name)
            shape = tuple(alloc.tensor_shape)
            dtype = mybir.dt.np(alloc.dtype)
            out_avals.append(jax.core.ShapedArray(shape, dtype))
            zero_shapes.append((shape, dtype))
    n_params = len(in_names)
    n_outs = len(out_names)
    all_names = in_names + out_names
    donate = tuple(range(n_params, n_params + n_outs))

    def _body(*args):
        outs = bass2jax._bass_exec_p.bind(
            *args,
            out_avals=tuple(out_avals),
            in_names=tuple(all_names),
            out_names=tuple(out_names),
            lowering_input_output_aliases=(),
            sim_require_finite=True,
            sim_require_nnan=True,
            nc=nc,
        )
        return tuple(outs)

    in_specs = (PartitionSpec("core"),) * (n_params + n_outs)
    out_specs = (PartitionSpec("core"),) * n_outs
    sharded = jax.jit(
        shard_map(_body, mesh=mesh, in_specs=in_specs, out_specs=out_specs,
                  check_rep=False),
        donate_argnums=donate, keep_unused=True)

    # background: prep + transfer while the main thread compiles
    xfer = {}

    def _transfer():
        try:
            in_maps = _prep_inputs(a)
            gl = []
            for i, name in enumerate(in_names):
                cat = np.concatenate([np.asarray(m[name]) for m in in_maps],
                                     axis=0)
                gl.append(jax.device_put(cat, sh))
            zs = [jax.device_put(
                np.zeros((n_cores * s[0], *s[1:]), d), sh)
                for (s, d) in zero_shapes]
            jax.block_until_ready(gl + zs)
            xfer["args"] = gl + zs
        except Exception as e:  # pragma: no cover
            xfer["err"] = e

    th = threading.Thread(target=_transfer)
    th.start()

    structs = []
    for i, name in enumerate(in_names):
        pass
    th_join_needed = True
    try:
        # avals for AOT lowering: need per-input global shapes/dtypes
        per0 = _prep_inputs(a)[0] if False else None
        # derive from BIR allocations instead (shape of ExternalInput)
        structs = []
        for alloc in nc.m.functions[0].allocations:
            if not isinstance(alloc, mybir.MemoryLocationSet):
                continue
            if alloc.kind == "ExternalInput":
                shape = tuple(alloc.tensor_shape)
                dtype = mybir.dt.np(alloc.dtype)
                structs.append(jax.ShapeDtypeStruct(
                    (n_cores * shape[0], *shape[1:]), dtype, sharding=sh))
        for (s, d) in zero_shapes:
            structs.append(jax.ShapeDtypeStruct(
                (n_cores * s[0], *s[1:]), d, sharding=sh))
        compiled = sharded.lower(*structs).compile()
        th.join()
        th_join_needed = False
        if "err" in xfer:
            raise xfer["err"]
        out_arrs = compiled(*xfer["args"])
    finally:
        if th_join_needed:
            th.join()
    res = []
    for c in range(n_cores):
        res.append({
            name: np.asarray(out_arrs[i]).reshape(
                n_cores, *out_avals[i].shape)[c]
            for i, name in enumerate(out_names)})
    return res


def _kernel_bass_inproc(a):
    global _NC
    sys.path.insert(0, "/opt/trn_rl_repo")
    from concourse import bass_utils

    if _NC is None:
        _NC = _build_bass()
    try:
        results = _run_fast(_NC, a)
    except Exception:
        traceback.print_exc()
        in_maps = _prep_inputs(a)
        results = bass_utils.run_bass_kernel_spmd(
            _NC, in_maps, core_ids=list(range(8))).results
    out = np.empty((B, S, D), np.float32)
    for core in range(8):
        b, lane = core // 4, core % 4
        rows = np.asarray(results[core]["out_rows"], np.float32)
        for r in range(4):
            out[b, (4 * r + lane) * 128:(4 * r + lane) * 128 + 128, :] = \
                rows[r * 128:(r + 1) * 128, :]
    return out


def _kernel_bass(a):
    """Run the Bass path in a subprocess with a clean jax env (the caller
    may have pinned JAX_PLATFORMS=cpu for the reference)."""
    import subprocess
    import tempfile

    td = tempfile.mkdtemp(prefix="akt_bass_")
    inp = os.path.join(td, "in.npz")
    outp = os.path.join(td, "out.npy")
    np.savez(inp, **a)
    env = dict(os.environ)
    env.pop("JAX_PLATFORMS", None)
    env.setdefault("JAX_COMPILATION_CACHE_DIR", "/root/.cache/jax_akt")
    env.setdefault("JAX_PERSISTENT_CACHE_MIN_ENTRY_SIZE_BYTES", "-1")
    env.setdefault("JAX_PERSISTENT_CACHE_MIN_COMPILE_TIME_SECS", "0")
    r = subprocess.run(
        [sys.executable, os.path.abspath(__file__), "--bass-child", inp, outp],
        env=env, capture_output=True, text=True, timeout=3000)
    if r.returncode != 0 or not os.path.exists(outp):
        raise RuntimeError(f"bass child failed:\n{r.stdout[-2000:]}\n{r.stderr[-4000:]}")
    return np.load(outp)


def kernel(**inputs):
    a = {k: np.asarray(inputs[k], np.float32) for k in ORDER}

    use_bass = os.environ.get("AKT_FORCE_NUMPY", "0") != "1"
    if use_bass:
        zeros = ["bk", "bv", "bo", "b1", "b2", "ln1_b", "ln2_b"]
        ones = ["ln1_s", "ln2_s"]
        if not all(np.all(a[k] == 0) for k in zeros):
            use_bass = False
        elif not all(np.all(a[k] == 1) for k in ones):
            use_bass = False

    if use_bass:
        try:
            return _kernel_bass(a)
        except Exception:
            traceback.print_exc()
    return _kernel_numpy(a)


if __name__ == "__main__":
    if len(sys.argv) == 4 and sys.argv[1] == "--bass-child":
        _z = np.load(sys.argv[2])
        _a = {k: _z[k] for k in _z.files}
        _o = _kernel_bass_inproc(_a)
        np.save(sys.argv[3], _o)


# revision 16
# speedup vs baseline: 1.2644x; 1.2644x over previous
"""AKT dense-transformer Bass kernel for 8 trn2 NeuronCores.

Sharding: data-parallel over batch (cores 0-3 = batch 0, cores 4-7 = batch 1).
Within a group each lane owns interleaved query row-blocks rg = 4*r + lane
(r = 0..3), so the causal-truncation width per local block r is
W_r = 512*(r+1), uniform across lanes (SPMD-safe).  Layer outputs are
republished group-wide with a transposed bf16 AllGather.  Matmuls run in bf16
(fp32 PSUM accumulation); the AKT softmax/decay chain runs in fp32 on DVE/ACT
using tensor_tensor_scan for the cumulative sum.

Falls back to a NumPy implementation if anything in the Bass path fails or the
inputs violate the fast path's assumptions (nonzero biases, non-unit LN).
"""

import os
import sys
import traceback

import numpy as np

B, S, D, H, DFF, L = 2, 2048, 512, 8, 2048, 3
DK = D // H  # 64
NLB = 4      # local row-blocks per core
P = 128

ORDER = ["s_embed_data", "sa_embed_data", "Wk", "bk", "Wv", "bv", "Wo", "bo",
         "gammas", "ln1_s", "ln1_b", "W1", "b1", "W2", "b2", "ln2_s", "ln2_b"]

# ---------------------------------------------------------------------------
# NumPy fallback
# ---------------------------------------------------------------------------

def _softmax(x, axis=-1):
    m = np.max(x, axis=axis, keepdims=True)
    e = np.exp(x - m)
    return e / np.sum(e, axis=axis, keepdims=True)


def _ln_np(x, s, b):
    m = x.mean(-1, keepdims=True)
    v = ((x - m) ** 2).mean(-1, keepdims=True)
    return (x - m) / np.sqrt(v + 1e-5) * s + b


_POS = None


def _pos():
    global _POS
    if _POS is None:
        ar = np.arange(S, dtype=np.float32)
        _POS = np.abs(ar[None, :] - ar[:, None])
    return _POS


def _attn_np(q, k, v, mask, zero_pad, gamma):
    scores = (q @ k.T) / np.float32(np.sqrt(DK))
    scores_ = _softmax(scores, axis=-1) * mask
    distcum = np.cumsum(scores_, axis=-1)
    disttotal = np.sum(scores_, axis=-1, keepdims=True)
    dist = np.sqrt(np.clip((disttotal - distcum) * _pos(), 0.0, None),
                   dtype=np.float32)
    g = -np.logaddexp(0.0, gamma)
    te = np.clip(np.exp(dist * g), 1e-5, 1e5).astype(np.float32)
    scores = scores * te
    scores = np.where(mask, scores, np.float32(-1e32))
    scores = _softmax(scores, axis=-1)
    if zero_pad:
        scores[0, :] = 0.0
    return scores @ v


def _layer_np(i, mask_k, query, keyx, values, apply_pos, a):
    q_all = query @ a["Wk"][i] + a["bk"][i]
    k_all = keyx @ a["Wk"][i] + a["bk"][i]
    v_all = values @ a["Wv"][i] + a["bv"][i]
    mask = np.tril(np.ones((S, S), bool), k=0 if mask_k == 1 else -1)
    o = np.empty((B, S, D), np.float32)
    for b in range(B):
        for h in range(H):
            sl = slice(h * DK, (h + 1) * DK)
            o[b, :, sl] = _attn_np(q_all[b, :, sl], k_all[b, :, sl],
                                   v_all[b, :, sl], mask, mask_k == 0,
                                   float(a["gammas"][i, h, 0, 0]))
    o = o @ a["Wo"][i] + a["bo"][i]
    out = _ln_np(query + o, a["ln1_s"][i], a["ln1_b"][i])
    if apply_pos:
        ff = np.maximum(out @ a["W1"][i] + a["b1"][i], 0.0) @ a["W2"][i] + a["b2"][i]
        out = _ln_np(out + ff, a["ln2_s"][i], a["ln2_b"][i])
    return out.astype(np.float32)


def _kernel_numpy(a):
    y = _layer_np(0, 1, a["sa_embed_data"], a["sa_embed_data"],
                  a["sa_embed_data"], True, a)
    x = _layer_np(1, 1, a["s_embed_data"], a["s_embed_data"],
                  a["s_embed_data"], False, a)
    x = _layer_np(2, 0, x, x, y, True, a)
    return x.astype(np.float32)


# ---------------------------------------------------------------------------
# Bass fast path
# ---------------------------------------------------------------------------

_NC = None


def _build_bass():
    import concourse.bass as bass
    import concourse.mybir as mybir
    import concourse.tile as tile

    # The Drain CTRL instruction supports only one sem wait on this walrus;
    # split the TileContext exit drain into one drain per wait.
    def _patched_drain_and_barrier(self, tick_clock, wait_clock):
        nc = self.nc
        drain_inst = nc.sync.drain()
        wait_clock.add_sem_waits(
            drain_inst.ins, tile.ScopedClock({None: tick_clock.global_clock}))
        si = drain_inst.ins.sync_info
        if si is not None and si.on_wait is not None and len(si.on_wait) > 1:
            waits = list(si.on_wait)
            si.on_wait = waits[:1]
            for w in waits[1:]:
                d2 = nc.sync.drain()
                si2 = d2.ins.sync_info
                if si2 is None:
                    d2.ins.sync_info = type(drain_inst.ins.sync_info)(
                        on_update=[], on_wait=[w])
                else:
                    si2.on_wait = [w]
        nc.all_engine_barrier()
        assert self.sems is not None
        popped = nc._tile_sem_poison_stack.pop()
        assert popped is self._sem_poison
        nc.clear_and_free_semaphores(list(self.sems.allocated().values()))
        nc.all_engine_barrier()

    tile.TileContext._drain_and_barrier = _patched_drain_and_barrier

    def _split_waits(nc):
        """walrus here allows only one sem-wait per instruction; hoist
        extras onto same-engine NoOps inserted just before."""
        n = 0
        for bb in nc.main_func.blocks:
            insts = bb.bb.instructions if hasattr(bb, "bb") else bb.instructions
            i = 0
            while i < len(insts):
                inst = insts[i]
                si = getattr(inst, "sync_info", None)
                if si is not None and si.on_wait is not None and len(si.on_wait) > 1:
                    waits = list(si.on_wait)
                    si.on_wait = waits[-1:]
                    for j, w in enumerate(waits[:-1]):
                        n += 1
                        nop = mybir.InstNoOp(
                            name=f"{inst.name}_wsplit{j}",
                            engine=inst.engine,
                            sync_info=mybir.SyncInfo(on_wait=[w], on_update=[]),
                            bass_nofuse=True,
                        )
                        insts.insert(i, nop)
                        i += 1
                i += 1
        return n

    dt = mybir.dt
    A = mybir.AluOpType
    AF = mybir.ActivationFunctionType
    f32, bf16 = dt.float32, dt.bfloat16

    nc = bass.Bass("TRN2", target_bir_lowering=False, debug=False,
                   num_devices=8)

    def din(name, shape, dty=f32):
        return nc.dram_tensor(name, shape, dty, kind="ExternalInput")

    saTo = din("saTo", [D, 512], bf16)
    sTo = din("sTo", [D, 512], bf16)
    sa_rows = din("sa_rows", [512, D])
    s_rows = din("s_rows", [512, D])
    # lane slices of the stacked replicated tensors (AllGathered on device)
    w5_sl = din("w5_sl", [2176, D], bf16)    # [wk0-2;wv0-2;wo0-2;w2_0;w2_2]/4
    w1_sl = din("w1_sl", [512, DFF], bf16)   # [w1_0;w1_2;saT;sT]/4
    g_in = din("gbc", [P, 24])
    mb_in = din("mbc", [P, 12])
    zp_in = din("zp", [1, 1])
    idb_in = din("idb", [P, P], bf16)
    idf_in = din("idf", [P, P])

    out_rows = nc.dram_tensor("out_rows", [512, D], f32, kind="ExternalOutput")

    RG = [[0, 1, 2, 3], [4, 5, 6, 7]]

    with tile.TileContext(nc) as tc:
        with (
            tc.tile_pool(name="c1", bufs=1) as c1,
            tc.tile_pool(name="xs", bufs=4) as xs,      # X^T source tiles
            tc.tile_pool(name="kt", bufs=4) as ktp,     # K^T tiles
            tc.tile_pool(name="ba", bufs=1) as ba,      # A/B/C big blocks
            tc.tile_pool(name="bp", bufs=1) as bp,      # P
            tc.tile_pool(name="fr", bufs=4) as fr,      # small frags
            tc.tile_pool(name="ro", bufs=1) as ro,      # row tiles
            tc.tile_pool(name="sm", bufs=8) as smp,     # [128,1] scalars
            tc.tile_pool(name="pss", bufs=1, space="PSUM") as pss,
            tc.tile_pool(name="pst", bufs=1, space="PSUM") as pst,
            tc.tile_pool(name="pso", bufs=1, space="PSUM") as pso,
            tc.tile_pool(name="psw", bufs=2, space="PSUM") as psw,
            tc.tile_pool(name="dram", bufs=2, space="DRAM") as dram,
        ):

            _tcnt = [0]

            def T(pool, shape, dty, tag):
                _tcnt[0] += 1
                return pool.tile(shape, dty, name=f"{tag}_{_tcnt[0]}", tag=tag)

            t0 = T(c1, [P, S], f32, "t0")
            nc.gpsimd.iota(t0[:], pattern=[[-1, S]], base=0,
                           channel_multiplier=1,
                           allow_small_or_imprecise_dtypes=True)
            gbc = T(c1, [P, 24], f32, "gbc")
            nc.sync.dma_start(gbc[:], g_in[:])
            mbc = T(c1, [P, 12], f32, "mbc")
            nc.sync.dma_start(mbc[:], mb_in[:])
            zp = T(c1, [1, 1], f32, "zp")
            nc.sync.dma_start(zp[:], zp_in[:])
            idb = T(c1, [P, P], bf16, "idb")
            nc.sync.dma_start(idb[:], idb_in[:])
            idf = T(c1, [P, P], f32, "idf")
            nc.sync.dma_start(idf[:], idf_in[:])

            eps = T(c1, [P, 1], f32, "eps")
            nc.gpsimd.memset(eps[:], 1e-5)
            w5b = dram.tile([2176, D], bf16, name="w5b", tag="w5b")
            nc.gpsimd.dma_start(w5b[:], w5_sl[:])
            w5g = dram.tile([8704, D], bf16, name="w5g", tag="w5g")
            nc.gpsimd.collective_compute(
                "AllGather", A.bypass, replica_groups=RG,
                ins=[w5b.opt()], outs=[w5g.opt()])
            w1b = dram.tile([512, DFF], bf16, name="w1b", tag="w1b")
            nc.gpsimd.dma_start(w1b[:], w1_sl[:])
            w1g = dram.tile([2048, DFF], bf16, name="w1g", tag="w1g")
            nc.gpsimd.collective_compute(
                "AllGather", A.bypass, replica_groups=RG,
                ins=[w1b.opt()], outs=[w1g.opt()])

            x1_rows = [T(c1, [P, D], f32, f"x1r{r}") for r in range(NLB)]
            x1T = [T(c1, [P, 512], bf16, f"x1T{i}") for i in range(4)]

            def tr_f32(dst_ap, src_ap):
                pt = T(pst, [P, P], f32, "pt")
                nc.tensor.transpose(pt[:], src_ap, idf[:])
                nc.vector.tensor_copy(dst_ap, pt[:])

            def ln_tile(xr, out_tag):
                s1 = T(smp, [P, 1], f32, "ls1")
                nc.vector.tensor_reduce(s1[:], xr[:],
                                        axis=mybir.AxisListType.X, op=A.add)
                nm = T(smp, [P, 1], f32, "lnm")
                nc.scalar.mul(nm[:], s1[:], -1.0 / D)
                xc = T(ro, [P, D], f32, "xc")
                nc.scalar.activation(xc[:], xr[:], AF.Identity, bias=nm[:])
                sq = T(psw, [P, D], f32, "w")
                s2_ = T(smp, [P, 1], f32, "ls2")
                nc.scalar.activation(sq[:], xc[:], AF.Square,
                                     accum_out=s2_[:])
                sd = T(smp, [P, 1], f32, "lsd")
                nc.scalar.activation(sd[:], s2_[:], AF.Sqrt, scale=1.0 / D,
                                     bias=eps[:])
                rs = T(smp, [P, 1], f32, "lrs")
                nc.vector.reciprocal(rs[:], sd[:])
                o = T(ro, [P, D], f32, out_tag)
                nc.vector.tensor_scalar(o[:], xc[:], rs[:], None, op0=A.mult)
                return o

            def layer(l, strict, v_xt, q_xt, qto, rows_in, apply_pos):
                # per-layer weights
                wk = [T(c1, [P, D], bf16, f"wk{i}") for i in range(4)]
                wv = [T(c1, [P, D], bf16, f"wv{i}") for i in range(4)]
                wo = [T(c1, [P, D], bf16, f"wo{i}") for i in range(4)]
                for i in range(4):
                    nc.sync.dma_start(
                        wk[i][:], w5g[l * 512 + i * P:l * 512 + (i + 1) * P, :])
                    nc.sync.dma_start(
                        wv[i][:],
                        w5g[(3 + l) * 512 + i * P:(3 + l) * 512 + (i + 1) * P, :])
                    nc.sync.dma_start(
                        wo[i][:],
                        w5g[(6 + l) * 512 + i * P:(6 + l) * 512 + (i + 1) * P, :])

                # V = Xv @ Wv : 16 k-block tiles [128, 512] bf16
                vt = []
                for kb in range(16):
                    ps = T(psw, [P, D], f32, "w")
                    for dc in range(4):
                        nc.tensor.matmul(
                            ps[:], v_xt[dc][:, kb * P:(kb + 1) * P], wv[dc][:],
                            start=(dc == 0), stop=(dc == 3))
                    v_ = T(c1, [P, D], bf16, f"v{kb}")
                    nc.scalar.copy(v_[:], ps[:])
                    vt.append(v_)

                # K^T = Wk^T @ Xq^T : 4 tiles [128, 2048] bf16
                kt = []
                for kc in range(4):
                    ps = T(pss, [P, S], f32, "s")
                    for dc in range(4):
                        for b4 in range(4):
                            nc.tensor.matmul(
                                ps[:, b4 * 512:(b4 + 1) * 512],
                                wk[dc][:, kc * P:(kc + 1) * P],
                                q_xt[dc][:, b4 * 512:(b4 + 1) * 512],
                                start=(dc == 0), stop=(dc == 3))
                    k_ = T(ktp, [P, S], bf16, "kt")
                    nc.vector.tensor_copy(k_[:], ps[:])
                    kt.append(k_)

                # Q^T own rows: 4 tiles [128, 512] bf16
                qtm = []
                for kc in range(4):
                    ps = T(psw, [P, D], f32, "w")
                    for dc in range(4):
                        nc.tensor.matmul(
                            ps[:], wk[dc][:, kc * P:(kc + 1) * P], qto[dc][:],
                            start=(dc == 0), stop=(dc == 3))
                    q_ = T(c1, [P, D], bf16, f"qtm{kc}")
                    nc.scalar.copy(q_[:], ps[:])
                    qtm.append(q_)

                oT = [T(c1, [P, 512], bf16, f"oT{i}") for i in range(4)]
                mboff = 4 if strict else 0
                for h in range(H):
                    ht, hp = h // 2, (h % 2) * DK
                    g_ap = gbc[:, (l * 8 + h):(l * 8 + h + 1)]
                    for r in range(NLB):
                        W_ = 512 * (r + 1)
                        nch = 4 * (r + 1)
                        mb_ap = mbc[:, mboff + r:mboff + r + 1]
                        rb_ap = mbc[:, 8 + r:8 + r + 1]

                        ps = T(pss, [P, S], f32, "s")
                        for b4 in range(4):
                            nc.tensor.matmul(
                                ps[:, b4 * 512:(b4 + 1) * 512],
                                qtm[ht][hp:hp + DK, r * P:(r + 1) * P],
                                kt[ht][hp:hp + DK, b4 * 512:(b4 + 1) * 512],
                                start=True, stop=True)

                        at = T(ba, [P, S], f32, "A")
                        bt = T(ba, [P, S], f32, "Bt")
                        ct = T(ba, [P, S], f32, "C")
                        z1 = T(smp, [P, 1], f32, "z1")
                        iz1 = T(smp, [P, 1], f32, "iz1")
                        z2 = T(smp, [P, 1], f32, "z2")
                        iz2 = T(smp, [P, 1], f32, "iz2")

                        nc.scalar.activation(at[:], ps[:], AF.Exp,
                                             scale=0.125, accum_out=z1[:])
                        nc.vector.reciprocal(iz1[:], z1[:])
                        nc.vector.scalar_tensor_tensor(
                            bt[:, :W_], t0[:, :W_], mb_ap, at[:, :W_],
                            op0=A.is_ge, op1=A.mult)
                        nc.vector.tensor_tensor_scan(
                            ct[:, :W_], bt[:, :W_], bt[:, :W_], 0.0,
                            op0=A.add, op1=A.max)
                        nc.vector.tensor_scalar(
                            bt[:, :W_], ct[:, :W_], ct[:, W_ - 1:W_], -1.0,
                            op0=A.subtract, op1=A.mult)
                        nc.vector.scalar_tensor_tensor(
                            ct[:, :W_], t0[:, :W_], rb_ap, bt[:, :W_],
                            op0=A.add, op1=A.mult)
                        nc.scalar.activation(bt[:, :W_], ct[:, :W_], AF.Sqrt,
                                             scale=iz1[:])
                        nc.scalar.activation(ct[:, :W_], bt[:, :W_], AF.Exp,
                                             scale=g_ap)
                        nc.vector.tensor_tensor(
                            bt[:, :W_], ps[:, :W_], ct[:, :W_], op=A.mult)
                        nc.scalar.activation(ct[:, :W_], bt[:, :W_], AF.Exp,
                                             scale=0.125)
                        nc.vector.scalar_tensor_tensor(
                            bt[:, :W_], t0[:, :W_], mb_ap, ct[:, :W_],
                            op0=A.is_ge, op1=A.mult, accum_out=z2[:])
                        nc.vector.tensor_scalar(
                            z2[:], z2[:], 1e-30, None, op0=A.add)
                        nc.vector.reciprocal(iz2[:], z2[:])
                        pb = T(bp, [P, S], bf16, "P")
                        nc.vector.tensor_scalar(
                            pb[:, :W_], bt[:, :W_], iz2[:], None, op0=A.mult)
                        if strict and r == 0:
                            nc.vector.tensor_scalar(
                                pb[0:1, :W_], pb[0:1, :W_], zp[:], None,
                                op0=A.mult)

                        po = T(pso, [DK, P], f32, "o")
                        for kc in range(nch):
                            pf = T(pst, [P, P], bf16, "pt")
                            nc.tensor.transpose(
                                pf[:], pb[:, kc * P:(kc + 1) * P], idb[:])
                            pfs = T(fr, [P, P], bf16, "pf")
                            nc.scalar.copy(pfs[:], pf[:])
                            nc.tensor.matmul(
                                po[:], vt[kc][:, h * DK:(h + 1) * DK], pfs[:],
                                start=(kc == 0), stop=(kc == nch - 1))
                        nc.vector.tensor_copy(
                            oT[ht][hp:hp + DK, r * P:(r + 1) * P], po[:])

                out_r = []
                for r in range(NLB):
                    ps = T(psw, [P, D], f32, "w")
                    for ch in range(4):
                        nc.tensor.matmul(
                            ps[:], oT[ch][:, r * P:(r + 1) * P], wo[ch][:],
                            start=(ch == 0), stop=(ch == 3))
                    xr = T(ro, [P, D], f32, "xr")
                    nc.vector.tensor_tensor(xr[:], ps[:], rows_in[r][:],
                                            op=A.add)
                    out_r.append(ln_tile(xr, f"lo{r}"))

                if not apply_pos:
                    return out_r

                w1 = [T(c1, [P, DFF], bf16, f"w1{i}") for i in range(4)]
                w2 = [T(c1, [P, D], bf16, f"w2{i}") for i in range(16)]
                w1base = 0 if l == 0 else 512
                w2base = 4608 if l == 0 else 6656
                for i in range(4):
                    nc.sync.dma_start(
                        w1[i][:], w1g[w1base + i * P:w1base + (i + 1) * P, :])
                for i in range(16):
                    nc.sync.dma_start(
                        w2[i][:], w5g[w2base + i * P:w2base + (i + 1) * P, :])
                fx = [T(c1, [P, 512], bf16, f"fxT{i}") for i in range(4)]
                for r in range(NLB):
                    for dc in range(4):
                        tr_f32(fx[dc][:, r * P:(r + 1) * P],
                               out_r[r][:, dc * P:(dc + 1) * P])
                fin = []
                for r in range(NLB):
                    po2 = T(pso, [P, D], f32, "o")
                    first = True
                    for b4 in range(4):
                        ph = T(psw, [P, 512], f32, "w")
                        for dc in range(4):
                            nc.tensor.matmul(
                                ph[:], fx[dc][:, r * P:(r + 1) * P],
                                w1[dc][:, b4 * 512:(b4 + 1) * 512],
                                start=(dc == 0), stop=(dc == 3))
                        hh = T(fr, [P, 512], bf16, "hh")
                        nc.scalar.activation(hh[:], ph[:], AF.Relu)
                        for ff in range(4):
                            pf = T(pst, [P, P], bf16, "pt")
                            nc.tensor.transpose(
                                pf[:], hh[:, ff * P:(ff + 1) * P], idb[:])
                            hf = T(fr, [P, P], bf16, "pf")
                            nc.scalar.copy(hf[:], pf[:])
                            nc.tensor.matmul(
                                po2[:], hf[:], w2[b4 * 4 + ff][:],
                                start=first, stop=(b4 == 3 and ff == 3))
                            first = False
                    xr2 = T(ro, [P, D], f32, "xr2")
                    nc.vector.tensor_tensor(xr2[:], po2[:], out_r[r][:],
                                            op=A.add)
                    fin.append(ln_tile(xr2, f"lf{r}"))
                return fin

            def publish(row_tiles, tag):
                ot = [T(c1, [P, 512], bf16, f"pub{tag}{i}")
                      for i in range(4)]
                for r in range(NLB):
                    for dc in range(4):
                        tr_f32(ot[dc][:, r * P:(r + 1) * P],
                               row_tiles[r][:, dc * P:(dc + 1) * P])
                bin_ = T(dram, [D, 512], bf16, f"bi{tag}")
                for dc in range(4):
                    nc.gpsimd.dma_start(bin_[dc * P:(dc + 1) * P, :],
                                        ot[dc][:])
                bout = T(dram, [4 * D, 512], bf16, f"bo{tag}")
                nc.gpsimd.collective_compute(
                    "AllGather", A.bypass, replica_groups=RG,
                    ins=[bin_.opt()], outs=[bout.opt()])
                return bout, ot

            def gath_to_xt(bout):
                xts = []
                for dc in range(4):
                    xt_t = T(xs, [P, S], bf16, "xsrc")
                    for r2 in range(4):
                        for c2 in range(4):
                            nc.sync.dma_start(
                                xt_t[:, (4 * r2 + c2) * P:(4 * r2 + c2 + 1) * P],
                                bout[c2 * 512 + dc * P:c2 * 512 + (dc + 1) * P,
                                     r2 * P:(r2 + 1) * P])
                    xts.append(xt_t)
                return xts

            # ---------------- main ----------------
            sa_r = [T(ro, [P, D], f32, f"rin{r}") for r in range(NLB)]
            for r in range(NLB):
                nc.sync.dma_start(sa_r[r][:], sa_rows[r * P:(r + 1) * P, :])
            saT_t = [T(xs, [P, S], bf16, "xsrc") for _ in range(4)]
            for dc in range(4):
                nc.sync.dma_start(
                    saT_t[dc][:], w1g[1024 + dc * P:1024 + (dc + 1) * P, :])
            qto0 = [T(c1, [P, 512], bf16, f"qto{i}") for i in range(4)]
            for dc in range(4):
                nc.sync.dma_start(qto0[dc][:], saTo[dc * P:(dc + 1) * P, :])

            y_rows = layer(0, False, saT_t, saT_t, qto0, sa_r, True)
            y_g, _ = publish(y_rows, "y")

            s_r = [T(ro, [P, D], f32, f"rin{r}") for r in range(NLB)]
            for r in range(NLB):
                nc.sync.dma_start(s_r[r][:], s_rows[r * P:(r + 1) * P, :])
            sT_t = [T(xs, [P, S], bf16, "xsrc") for _ in range(4)]
            for dc in range(4):
                nc.sync.dma_start(
                    sT_t[dc][:], w1g[1536 + dc * P:1536 + (dc + 1) * P, :])
            qto1 = [T(c1, [P, 512], bf16, f"qto{i}") for i in range(4)]
            for dc in range(4):
                nc.sync.dma_start(qto1[dc][:], sTo[dc * P:(dc + 1) * P, :])

            x_rows = layer(1, False, sT_t, sT_t, qto1, s_r, False)
            for r in range(NLB):
                nc.vector.tensor_copy(x1_rows[r][:], x_rows[r][:])
            x_g, x_ot = publish(x_rows, "x")
            for dc in range(4):
                nc.vector.tensor_copy(x1T[dc][:], x_ot[dc][:])

            yT_t = gath_to_xt(y_g)
            xT_t = gath_to_xt(x_g)
            f_rows = layer(2, True, yT_t, xT_t, x1T, x1_rows, True)
            for r in range(NLB):
                nc.sync.dma_start(out_rows[r * P:(r + 1) * P, :], f_rows[r][:])

    _split_waits(nc)
    return nc


def _prep_inputs(a):
    import ml_dtypes
    bf16 = ml_dtypes.bfloat16

    g = -np.logaddexp(0.0, a["gammas"][:, :, 0, 0]).astype(np.float32)
    gb = np.zeros((128, 24), np.float32)
    for l in range(3):
        for h in range(8):
            gb[:, l * 8 + h] = g[l, h]
    ident = np.eye(128, dtype=np.float32)

    w5 = np.concatenate([a["Wk"][0], a["Wk"][1], a["Wk"][2],
                         a["Wv"][0], a["Wv"][1], a["Wv"][2],
                         a["Wo"][0], a["Wo"][1], a["Wo"][2],
                         a["W2"][0], a["W2"][2]], axis=0).astype(bf16)

    in_maps = []
    for core in range(8):
        b, lane = core // 4, core % 4
        rows_idx = np.concatenate(
            [np.arange((4 * r + lane) * 128, (4 * r + lane) * 128 + 128)
             for r in range(4)])
        sa_b = a["sa_embed_data"][b]
        s_b = a["s_embed_data"][b]
        mb = np.zeros((128, 12), np.float32)
        for r in range(4):
            rb = (4 * r + lane) * 128.0
            mb[:, r] = -rb
            mb[:, 4 + r] = -rb + 1.0
            mb[:, 8 + r] = rb
        w1stack = np.concatenate(
            [a["W1"][0], a["W1"][2],
             np.ascontiguousarray(sa_b.T), np.ascontiguousarray(s_b.T)],
            axis=0).astype(bf16)
        m = {
            "w5_sl": w5[lane * 2176:(lane + 1) * 2176],
            "w1_sl": np.ascontiguousarray(
                w1stack[lane * 512:(lane + 1) * 512]),
            "saTo": np.ascontiguousarray(sa_b[rows_idx].T).astype(bf16),
            "sTo": np.ascontiguousarray(s_b[rows_idx].T).astype(bf16),
            "sa_rows": np.ascontiguousarray(sa_b[rows_idx]),
            "s_rows": np.ascontiguousarray(s_b[rows_idx]),
            "gbc": gb,
            "mbc": mb,
            "zp": np.array([[0.0 if lane == 0 else 1.0]], np.float32),
            "idb": ident.astype(bf16),
            "idf": ident,
        }
        in_maps.append(m)
    return in_maps


def _run_fast(nc, a):
    """Fork of bass2jax.run_bass_via_pjrt's 8-core path that overlaps the
    host->device input transfer (background thread) with XLA/walrus compile
    (AOT .lower().compile() needs only avals)."""
    import threading
    import jax
    import jax.numpy as jnp
    from jax.sharding import Mesh, PartitionSpec, NamedSharding
    from jax.experimental.shard_map import shard_map
    import concourse.mybir as mybir
    from concourse import bass2jax

    bass2jax.install_neuronx_cc_hook()
    n_cores = 8
    devices = jax.devices()[:n_cores]
    assert len(devices) == n_cores
    mesh = Mesh(np.asarray(devices), ("core",))
    sh = NamedSharding(mesh, PartitionSpec("core"))

    pname = nc.partition_id_tensor.name if nc.partition_id_tensor else None
    in_names, out_names, out_avals = [], [], []
    zero_shapes = []
    for alloc in nc.m.functions[0].allocations:
        if not isinstance(alloc, mybir.MemoryLocationSet):
            continue
        name = alloc.memorylocations[0].name
        if alloc.kind == "ExternalInput":
            if name != pname:
                in_names.append(name)
        elif alloc.kind == "ExternalOutput":
            out_names.append(name)
            shape = tuple(alloc.tensor_shape)
            dtype = mybir.dt.np(alloc.dtype)
            out_avals.append(jax.core.ShapedArray(shape, dtype))
            zero_shapes.append((shape, dtype))
    n_params = len(in_names)
    n_outs = len(out_names)
    all_names = in_names + out_names + ([pname] if pname else [])
    donate = tuple(range(n_params, n_params + n_outs))

    def _body(*args):
        operands = list(args)
        if pname:
            operands.append(bass2jax.partition_id_tensor())
        outs = bass2jax._bass_exec_p.bind(
            *operands,
            out_avals=tuple(out_avals),
            in_names=tuple(all_names),
            out_names=tuple(out_names),
            lowering_input_output_aliases=(),
            sim_require_finite=True,
            sim_require_nnan=True,
            nc=nc,
        )
        return tuple(outs)

    in_specs = (PartitionSpec("core"),) * (n_params + n_outs)
    out_specs = (PartitionSpec("core"),) * n_outs
    sharded = jax.jit(
        shard_map(_body, mesh=mesh, in_specs=in_specs, out_specs=out_specs,
                  check_rep=False),
        donate_argnums=donate, keep_unused=True)

    # background: prep + transfer while the main thread compiles
    xfer = {}

    def _transfer():
        try:
            in_maps = _prep_inputs(a)
            gl = []
            for i, name in enumerate(in_names):
                cat = np.concatenate([np.asarray(m[name]) for m in in_maps],
                                     axis=0)
                gl.append(jax.device_put(cat, sh))
            zs = [jax.device_put(
                np.zeros((n_cores * s[0], *s[1:]), d), sh)
                for (s, d) in zero_shapes]
            jax.block_until_ready(gl + zs)
            xfer["args"] = gl + zs
        except Exception as e:  # pragma: no cover
            xfer["err"] = e

    th = threading.Thread(target=_transfer)
    th.start()

    structs = []
    for i, name in enumerate(in_names):
        pass
    th_join_needed = True
    try:
        # avals for AOT lowering: need per-input global shapes/dtypes
        per0 = _prep_inputs(a)[0] if False else None
        # derive from BIR allocations instead (shape of ExternalInput)
        structs = []
        for alloc in nc.m.functions[0].allocations:
            if not isinstance(alloc, mybir.MemoryLocationSet):
                continue
            if alloc.kind == "ExternalInput" and \
                    alloc.memorylocations[0].name != pname:
                shape = tuple(alloc.tensor_shape)
                dtype = mybir.dt.np(alloc.dtype)
                structs.append(jax.ShapeDtypeStruct(
                    (n_cores * shape[0], *shape[1:]), dtype, sharding=sh))
        for (s, d) in zero_shapes:
            structs.append(jax.ShapeDtypeStruct(
                (n_cores * s[0], *s[1:]), d, sharding=sh))
        compiled = sharded.lower(*structs).compile()
        th.join()
        th_join_needed = False
        if "err" in xfer:
            raise xfer["err"]
        out_arrs = compiled(*xfer["args"])
    finally:
        if th_join_needed:
            th.join()
    res = []
    for c in range(n_cores):
        res.append({
            name: np.asarray(out_arrs[i]).reshape(
                n_cores, *out_avals[i].shape)[c]
            for i, name in enumerate(out_names)})
    return res


def _kernel_bass_inproc(a):
    global _NC
    sys.path.insert(0, "/opt/trn_rl_repo")
    from concourse import bass_utils

    if _NC is None:
        _NC = _build_bass()
    in_maps = _prep_inputs(a)
    results = bass_utils.run_bass_kernel_spmd(
        _NC, in_maps, core_ids=list(range(8))).results
    out = np.empty((B, S, D), np.float32)
    for core in range(8):
        b, lane = core // 4, core % 4
        rows = np.asarray(results[core]["out_rows"], np.float32)
        for r in range(4):
            out[b, (4 * r + lane) * 128:(4 * r + lane) * 128 + 128, :] = \
                rows[r * 128:(r + 1) * 128, :]
    return out


def _kernel_bass(a):
    """Run the Bass path in a subprocess with a clean jax env (the caller
    may have pinned JAX_PLATFORMS=cpu for the reference)."""
    import subprocess
    import tempfile

    td = tempfile.mkdtemp(prefix="akt_bass_")
    inp = os.path.join(td, "in.npz")
    outp = os.path.join(td, "out.npy")
    np.savez(inp, **a)
    env = dict(os.environ)
    env.pop("JAX_PLATFORMS", None)
    env.setdefault("JAX_COMPILATION_CACHE_DIR", "/root/.cache/jax_akt")
    env.setdefault("JAX_PERSISTENT_CACHE_MIN_ENTRY_SIZE_BYTES", "-1")
    env.setdefault("JAX_PERSISTENT_CACHE_MIN_COMPILE_TIME_SECS", "0")
    r = subprocess.run(
        [sys.executable, os.path.abspath(__file__), "--bass-child", inp, outp],
        env=env, capture_output=True, text=True, timeout=3000)
    if r.returncode != 0 or not os.path.exists(outp):
        raise RuntimeError(f"bass child failed:\n{r.stdout[-2000:]}\n{r.stderr[-4000:]}")
    return np.load(outp)


def kernel(**inputs):
    a = {k: np.asarray(inputs[k], np.float32) for k in ORDER}

    use_bass = os.environ.get("AKT_FORCE_NUMPY", "0") != "1"
    if use_bass:
        zeros = ["bk", "bv", "bo", "b1", "b2", "ln1_b", "ln2_b"]
        ones = ["ln1_s", "ln2_s"]
        if not all(np.all(a[k] == 0) for k in zeros):
            use_bass = False
        elif not all(np.all(a[k] == 1) for k in ones):
            use_bass = False

    if use_bass:
        try:
            return _kernel_bass(a)
        except Exception:
            traceback.print_exc()
    return _kernel_numpy(a)


if __name__ == "__main__":
    if len(sys.argv) == 4 and sys.argv[1] == "--bass-child":
        _z = np.load(sys.argv[2])
        _a = {k: _z[k] for k in _z.files}
        _o = _kernel_bass_inproc(_a)
        np.save(sys.argv[3], _o)
